# revision 1
# baseline (speedup 1.0000x reference)
"""EnhancedGCN (2x GCNConv + GAT head + log_softmax) on 8 Trainium2 NeuronCores.

Strategy (graph/data parallel, per sharding hint):
- Nodes sharded contiguously across 8 cores (12500 each); within each core,
  nodes are permuted by descending in-degree so fixed-slot edge buffers pad well.
- Each layer: local matmul (features bf16) -> scale rows by dinv[src] -> AllGather
  the transformed node table -> each core pull-gathers its in-edges' source rows
  with one indirect DMA per slot-group -> tree-reduce slots -> scale by dinv[dst],
  bias, relu. GAT head gathers [g(14) | a_src(2)] rows, does segment-softmax with
  fixed slots, then log_softmax.
- All graph preprocessing (self-loops, degrees, CSR bucketing, slot layout,
  gather indices) is host-side numpy from edge_index only.
"""
import sys, os
sys.path.insert(0, '/opt/trn_rl_repo')

import numpy as np
import ml_dtypes

import concourse.bass as bass
import concourse.bacc as bacc
from concourse import mybir
import concourse.tile as tile
from concourse.masks import make_identity
from concourse.bass_utils import run_bass_kernel_spmd

BF = ml_dtypes.bfloat16
F32 = mybir.dt.float32
BF16 = mybir.dt.bfloat16
I32 = mybir.dt.int32
AL = mybir.AluOpType
AF = mybir.ActivationFunctionType
AX = mybir.AxisListType

N = 100000
NC = 8
NPC = N // NC            # 12500 nodes per core
P = 128
NT = 98                  # dst tiles per core (98*128 = 12544 ranks, 44 dummy)
NRANKS = NT * P          # 12544
TROWS = NRANKS + 1       # +1 zero row per core shard
F_IN = 1433
F_PAD = 1536             # 12 * 128
KC = F_PAD // P          # 12 k-chunks
HID = 64
CLS = 7
HEADS = 2
GW = 16                  # gat row: g(14) + a_src(2)
SLOTCAP = 128            # max slots (Tg*Dg) per partition per gather group
MAXT = 12                # max tiles per group
GCHUNK = int(os.environ.get("KGCHUNK", "16"))  # idx columns per indirect DMA call
NSWQ = int(os.environ.get("KNSWQ", "4"))       # SWDGE queues to round-robin
KACC = int(os.environ.get("KKACC", "1"))       # gather accumulate passes (GCN layers, f32 tables only)

_cache = {}


def _make_groups(D_t):
    """groups: (t0, T, Dg, gat_base, Dg2, npass, gcn_base)."""
    groups = []
    t0 = 0
    gat_base = 0
    gcn_base = 0
    while t0 < NT:
        Dg = int(D_t[t0])
        Dg2 = (Dg + KACC - 1) // KACC
        npass = (Dg + Dg2 - 1) // Dg2
        T = 1
        while (t0 + T < NT) and (T < MAXT) and (Dg * (T + 1) <= SLOTCAP):
            T += 1
        groups.append((t0, T, Dg, gat_base, Dg2, npass, gcn_base))
        gat_base += T * Dg
        gcn_base += T * Dg2 * npass
        t0 += T
    return groups, gat_base, gcn_base


def _preprocess(edge_index):
    src0 = edge_index[0].astype(np.int64)
    dst0 = edge_index[1].astype(np.int64)
    loop = np.arange(N, dtype=np.int64)
    src = np.concatenate([src0, loop])
    dst = np.concatenate([dst0, loop])
    deg = np.bincount(dst, minlength=N)
    dinv = (1.0 / np.sqrt(np.maximum(deg, 1))).astype(np.float32)
    dinv[deg == 0] = 0.0

    perms = []
    rank_of = np.empty(N, np.int64)
    degs_sorted = np.empty((NC, NPC), np.int64)
    for k in range(NC):
        ld = deg[k * NPC:(k + 1) * NPC]
        perm = np.argsort(-ld, kind='stable')
        perms.append(perm)
        rank_of[k * NPC + perm] = np.arange(NPC)
        degs_sorted[k] = ld[perm]
    tpos = (np.arange(N) // NPC) * TROWS + rank_of  # node -> table row

    dpad = np.zeros((NC, NRANKS), np.int64)
    dpad[:, :NPC] = degs_sorted
    D_t = np.maximum(dpad[:, 0:NRANKS:P].max(axis=0), 1)  # [NT]
    groups, S_gat, S_gcn = _make_groups(D_t)

    owner = dst // NPC
    esp_all = tpos[src]
    gidx = np.empty((NC, P, S_gat), np.int32)
    gidx2 = np.empty((NC, P, S_gcn), np.int32)
    dinv_arr = np.zeros((NC, P, NT), np.float32)
    for k in range(NC):
        sel = owner == k
        esp = esp_all[sel]
        er = rank_of[dst[sel]]
        order = np.argsort(er, kind='stable')
        esp = esp[order]
        cnt = np.bincount(er[order], minlength=NRANKS)
        roff = np.concatenate([[0], np.cumsum(cnt)])
        zrow = k * TROWS + NRANKS
        idxk = np.full((P, S_gat), zrow, np.int32)
        idxk2 = np.full((P, S_gcn), zrow, np.int32)
        for (t0, T, Dg, base, Dg2, npass, base2) in groups:
            for tt in range(T):
                t = t0 + tt
                ranks = t * P + np.arange(P)
                c = cnt[ranks]
                sidx = roff[ranks][:, None] + np.arange(Dg)[None, :]
                mask = np.arange(Dg)[None, :] < c[:, None]
                vals = np.where(mask, esp[np.minimum(sidx, len(esp) - 1)], zrow)
                idxk[:, base + tt * Dg: base + (tt + 1) * Dg] = vals
                # GCN layout: edge s -> pass s // Dg2, pos s % Dg2
                for pp in range(npass):
                    lo = pp * Dg2
                    w = min(Dg2, Dg - lo)
                    dst_cols = base2 + pp * T * Dg2 + tt * Dg2
                    idxk2[:, dst_cols: dst_cols + w] = vals[:, lo: lo + w]
        gidx[k] = idxk
        gidx2[k] = idxk2
        dvp = np.zeros(NRANKS, np.float32)
        dvp[:NPC] = dinv[k * NPC:(k + 1) * NPC][perms[k]]
        dinv_arr[k] = dvp.reshape(NT, P).T
    return perms, groups, S_gat, S_gcn, gidx, gidx2, dinv_arr


def _tree_reduce(nc, view, Dg):
    """In-place pairwise sum over the slot axis of view [128, T, D, F*]."""
    dd = Dg
    while dd > 1:
        h = dd // 2
        nd = dd - h
        nc.vector.tensor_add(
            out=view[:, :, 0:h, :], in0=view[:, :, 0:h, :], in1=view[:, :, nd:dd, :]
        )
        dd = nd


import os
DBG = os.environ.get("KDBG", "0") == "1"
DBG2 = os.environ.get("KDBG2", "0") == "1"


def _build(groups, S_gat, S_gcn):
    nc = bacc.Bacc("TRN2", target_bir_lowering=False, debug=False, num_devices=NC,
                   num_swdge_queues=NSWQ)
    qrr = [0]

    def gq(inst):
        if NSWQ > 1:
            i = qrr[0] % NSWQ
            qrr[0] += 1
            if i:
                inst.ins.queue = f"qPoolDynamic{i}"
        return inst

    xT = nc.dram_tensor("xT", [F_PAD, NRANKS], BF16, kind="ExternalInput")
    w1 = nc.dram_tensor("w1", [P, KC * HID], BF16, kind="ExternalInput")
    w2 = nc.dram_tensor("w2", [HID, HID], BF16, kind="ExternalInput")
    wg = nc.dram_tensor("wg", [HID, 18], BF16, kind="ExternalInput")
    b1 = nc.dram_tensor("b1", [P, HID], BF16, kind="ExternalInput")
    b2 = nc.dram_tensor("b2", [P, HID], BF16, kind="ExternalInput")
    bg = nc.dram_tensor("bg", [P, 14], F32, kind="ExternalInput")
    dinv = nc.dram_tensor("dinv", [P, NT], F32, kind="ExternalInput")
    gidx = nc.dram_tensor("gidx", [P, S_gat], I32, kind="ExternalInput")
    gidx2 = (nc.dram_tensor("gidx2", [P, S_gcn], I32, kind="ExternalInput")
             if KACC > 1 else None)
    out = nc.dram_tensor("out", [NRANKS, 14], F32, kind="ExternalOutput")
    if DBG:
        dbg_t1 = nc.dram_tensor("dbg_t1", [NC * TROWS, 32], F32, kind="ExternalOutput")
        dbg_t2 = nc.dram_tensor("dbg_t2", [NC * TROWS, 32], F32, kind="ExternalOutput")
        dbg_t3 = nc.dram_tensor("dbg_t3", [NC * TROWS, 8], F32, kind="ExternalOutput")
        dbg_h1 = nc.dram_tensor("dbg_h1", [HID, NRANKS], BF16, kind="ExternalOutput")
    if DBG2:
        dbg_s3 = nc.dram_tensor("dbg_s3", [P, S_gat * 8], F32, kind="ExternalOutput")
        dbg_e = nc.dram_tensor("dbg_e", [P, S_gat * 2], F32, kind="ExternalOutput")

    with tile.TileContext(nc) as tc:
        with tc.tile_pool(name="const", bufs=1) as cp, \
             tc.tile_pool(name="io2", bufs=2) as io2, \
             tc.tile_pool(name="io3", bufs=3) as io3, \
             tc.tile_pool(name="psmm", bufs=4, space="PSUM") as psmm, \
             tc.tile_pool(name="pstp", bufs=4, space="PSUM") as pstp, \
             tc.tile_pool(name="dram", bufs=1, space="DRAM") as dram:

            ident = cp.tile([P, P], F32, tag="ident")
            make_identity(nc, ident[:])
            w1t = cp.tile([P, KC * HID], BF16, tag="w1t")
            nc.sync.dma_start(w1t[:], w1[:])
            w2t = cp.tile([HID, HID], BF16, tag="w2t")
            nc.sync.dma_start(w2t[:], w2[:])
            wgt = cp.tile([HID, 18], BF16, tag="wgt")
            nc.sync.dma_start(wgt[:], wg[:])
            b1t = cp.tile([P, HID], BF16, tag="b1t")
            nc.sync.dma_start(b1t[:], b1[:])
            b2t = cp.tile([P, HID], BF16, tag="b2t")
            nc.sync.dma_start(b2t[:], b2[:])
            bgt = cp.tile([P, 14], F32, tag="bgt")
            nc.sync.dma_start(bgt[:], bg[:])
            dvt = cp.tile([P, NT], F32, tag="dvt")
            nc.sync.dma_start(dvt[:], dinv[:])
            idxt = cp.tile([P, S_gat], I32, tag="idxt")
            nc.sync.dma_start(idxt[:], gidx[:])
            if KACC > 1:
                idxt2 = cp.tile([P, S_gcn], I32, tag="idxt2")
                nc.sync.dma_start(idxt2[:], gidx2[:])
            else:
                idxt2 = idxt

            zt = cp.tile([1, 32], F32, tag="zt")
            nc.vector.memset(zt[:], 0.0)
            gpad = cp.tile([1, GW], BF16, tag="gpad")
            nc.vector.memset(gpad[:, 0:14], 0.0)
            nc.vector.memset(gpad[:, 14:GW], -1e30)

            shard1 = dram.tile([TROWS, 32], F32, tag="shard1")
            shard2 = dram.tile([TROWS, 32], F32, tag="shard2")
            shard3 = dram.tile([TROWS, 8], F32, tag="shard3")
            table1 = dram.tile([NC * TROWS, 32], F32, tag="table1", addr_space="Shared")
            table2 = dram.tile([NC * TROWS, 32], F32, tag="table2", addr_space="Shared")
            table3 = dram.tile([NC * TROWS, 8], F32, tag="table3", addr_space="Shared")

            adst = cp.tile([P, NT * 2], F32, tag="adst")
            oacc = cp.tile([P, NT * 14], F32, tag="oacc")
            h1T = cp.tile([HID, NRANKS], BF16, tag="h1T")
            h2T = cp.tile([HID, NRANKS], BF16, tag="h2T")

            # ---------------- Stage A: h1pre.T = W1.T @ x.T, scale, pack shard1
            acc1 = cp.tile([P, NT * HID], BF16, tag="acc1")
            mg_edges = list(range(0, NRANKS, 2048))
            for c0 in mg_edges:
                w = min(2048, NRANKS - c0)
                nrb = (w + 511) // 512
                npair = (nrb + 1) // 2
                psums = []
                for k in range(KC):
                    xt = io3.tile([P, 2048], BF16, tag="xt")
                    nc.sync.dma_start(xt[:, :w], xT[k * P:(k + 1) * P, c0:c0 + w])
                    for pr in range(npair):
                        if k == 0:
                            psums.append(
                                psmm.tile([P, 512], F32, tag="mm", name="psmm_a")
                            )
                        # even rowblock -> col group 0-1 (psum partitions 0-63)
                        rb0 = pr * 2
                        rw0 = min(512, w - rb0 * 512)
                        nc.tensor.matmul(
                            out=psums[pr][0:HID, :rw0],
                            lhsT=w1t[:, k * HID:(k + 1) * HID],
                            rhs=xt[:, rb0 * 512: rb0 * 512 + rw0],
                            start=(k == 0), stop=(k == KC - 1),
                            tile_position=(0, 0),
                        )
                        rb1 = pr * 2 + 1
                        if rb1 < nrb:
                            rw1 = min(512, w - rb1 * 512)
                            nc.tensor.matmul(
                                out=psums[pr][HID:2 * HID, :rw1],
                                lhsT=w1t[:, k * HID:(k + 1) * HID],
                                rhs=xt[:, rb1 * 512: rb1 * 512 + rw1],
                                start=(k == 0), stop=(k == KC - 1),
                                tile_position=(0, HID),
                            )
                for pr in range(npair):
                    for half in range(2):
                        rb = pr * 2 + half
                        if rb >= nrb:
                            continue
                        rw = min(512, w - rb * 512)
                        stg = io2.tile([HID, 512], F32, tag="stg")
                        nc.vector.tensor_copy(
                            out=stg[:, :rw],
                            in_=psums[pr][half * HID:(half + 1) * HID, :rw],
                        )
                        for b in range(rw // P):
                            rt = (c0 + rb * 512 + b * P) // P
                            tp = pstp.tile([P, HID], F32, tag="tp")
                            nc.tensor.transpose(
                                out=tp[:], in_=stg[:, b * P:(b + 1) * P],
                                identity=ident[0:HID, 0:HID],
                            )
                            nc.vector.tensor_scalar(
                                out=acc1[:, rt * HID:(rt + 1) * HID],
                                in0=tp[:], scalar1=dvt[:, rt:rt + 1], scalar2=None,
                                op0=AL.mult,
                            )
            nc.sync.dma_start(
                out=shard1[:NRANKS, :].rearrange("(rt p) c -> p rt c", p=P),
                in_=acc1[:].bitcast(F32).rearrange("p (rt c) -> p rt c", rt=NT),
            )
            nc.sync.dma_start(out=shard1[NRANKS:TROWS, :], in_=zt[:])

            # ---------------- AllGather 1
            nc.gpsimd.collective_compute(
                "AllGather", AL.bypass, replica_groups=[list(range(NC))],
                ins=[shard1.opt()], outs=[table1.opt()],
            )

            # ---------------- GCN aggregation helper
            def gcn_aggregate(table, btile, hT):
                for (t0, T, Dg, base, Dg2, npass, base2) in groups:
                    slot = io2.tile([P, SLOTCAP * 32], F32, tag="slot", bufs=3)
                    for pp in range(npass):
                        op = AL.bypass if pp == 0 else AL.add
                        pbase = base2 + pp * T * Dg2
                        for c0 in range(0, T * Dg2, GCHUNK):
                            cw = min(GCHUNK, T * Dg2 - c0)
                            gq(nc.gpsimd.indirect_dma_start(
                                out=slot[:, c0 * 32: (c0 + cw) * 32],
                                out_offset=None,
                                in_=table.opt(),
                                in_offset=bass.IndirectOffsetOnAxis(
                                    ap=idxt2[:, pbase + c0: pbase + c0 + cw], axis=0
                                ),
                                compute_op=op,
                            ))
                    sv = slot[:, : T * Dg2 * 32].bitcast(BF16).rearrange(
                        "p (t d f) -> p t d f", t=T, d=Dg2
                    )
                    _tree_reduce(nc, sv, Dg2)
                    hbuf = io2.tile([P, MAXT * HID], F32, tag="hbuf")
                    for tt in range(T):
                        nc.vector.scalar_tensor_tensor(
                            out=hbuf[:, tt * HID:(tt + 1) * HID],
                            in0=sv[:, tt, 0, :],
                            scalar=dvt[:, t0 + tt: t0 + tt + 1],
                            in1=btile[:],
                            op0=AL.mult, op1=AL.add,
                        )
                    nc.scalar.activation(
                        out=hbuf[:, : T * HID], in_=hbuf[:, : T * HID], func=AF.Relu
                    )
                    for tt in range(T):
                        tph = pstp.tile([HID, P], F32, tag="tp")
                        nc.tensor.transpose(
                            out=tph[:], in_=hbuf[:, tt * HID:(tt + 1) * HID],
                            identity=ident[:],
                        )
                        nc.vector.tensor_copy(
                            out=hT[:, (t0 + tt) * P:(t0 + tt + 1) * P], in_=tph[:]
                        )

            # ---------------- Stage D: L1 aggregation -> h1T
            gcn_aggregate(table1, b1t, h1T)

            # ---------------- Stage E: L2 matmul + shard2
            acc2 = cp.tile([P, NT * HID], BF16, tag="acc2")
            for c0 in range(0, NRANKS, 512):
                w = min(512, NRANKS - c0)
                ps2 = psmm.tile([HID, 512], F32, tag="mm")
                nc.tensor.matmul(
                    out=ps2[:, :w], lhsT=w2t[:], rhs=h1T[:, c0:c0 + w],
                    start=True, stop=True,
                )
                stg = io2.tile([HID, 512], F32, tag="stg")
                nc.vector.tensor_copy(out=stg[:, :w], in_=ps2[:, :w])
                for b in range(w // P):
                    rt = (c0 + b * P) // P
                    tp = pstp.tile([P, HID], F32, tag="tp")
                    nc.tensor.transpose(
                        out=tp[:], in_=stg[:, b * P:(b + 1) * P],
                        identity=ident[0:HID, 0:HID],
                    )
                    nc.vector.tensor_scalar(
                        out=acc2[:, rt * HID:(rt + 1) * HID],
                        in0=tp[:], scalar1=dvt[:, rt:rt + 1], scalar2=None,
                        op0=AL.mult,
                    )
            nc.sync.dma_start(
                out=shard2[:NRANKS, :].rearrange("(rt p) c -> p rt c", p=P),
                in_=acc2[:].bitcast(F32).rearrange("p (rt c) -> p rt c", rt=NT),
            )
            nc.sync.dma_start(out=shard2[NRANKS:TROWS, :], in_=zt[:])

            # ---------------- AllGather 2
            nc.gpsimd.collective_compute(
                "AllGather", AL.bypass, replica_groups=[list(range(NC))],
                ins=[shard2.opt()], outs=[table2.opt()],
            )

            # ---------------- Stage G: L2 aggregation -> h2T
            gcn_aggregate(table2, b2t, h2T)

            # ---------------- Stage H: GAT node transforms + shard3
            acc3 = cp.tile([P, NT * GW], BF16, tag="acc3")
            for c0 in range(0, NRANKS, 512):
                w = min(512, NRANKS - c0)
                ps3 = psmm.tile([18, 512], F32, tag="mm")
                nc.tensor.matmul(
                    out=ps3[:, :w], lhsT=wgt[:], rhs=h2T[:, c0:c0 + w],
                    start=True, stop=True,
                )
                stg = io2.tile([18, 512], F32, tag="stg3")
                nc.vector.tensor_copy(out=stg[:, :w], in_=ps3[:, :w])
                for b in range(w // P):
                    rt = (c0 + b * P) // P
                    tp = pstp.tile([P, 18], F32, tag="tp")
                    nc.tensor.transpose(
                        out=tp[:], in_=stg[:, b * P:(b + 1) * P],
                        identity=ident[0:18, 0:18],
                    )
                    nc.vector.tensor_copy(
                        out=acc3[:, rt * GW: rt * GW + GW], in_=tp[:, 0:GW]
                    )
                    nc.vector.tensor_copy(
                        out=adst[:, rt * 2: rt * 2 + 2], in_=tp[:, 16:18]
                    )
            nc.sync.dma_start(
                out=shard3[:NRANKS, :].rearrange("(rt p) c -> p rt c", p=P),
                in_=acc3[:].bitcast(F32).rearrange("p (rt c) -> p rt c", rt=NT),
            )
            nc.sync.dma_start(out=shard3[NRANKS:TROWS, :], in_=gpad[:].bitcast(F32))

            # ---------------- AllGather 3
            nc.gpsimd.collective_compute(
                "AllGather", AL.bypass, replica_groups=[list(range(NC))],
                ins=[shard3.opt()], outs=[table3.opt()],
            )

            if DBG:
                nc.sync.dma_start(out=dbg_t1[:], in_=table1.opt())
                nc.sync.dma_start(out=dbg_t2[:], in_=table2.opt())
                nc.sync.dma_start(out=dbg_t3[:], in_=table3.opt())
                nc.sync.dma_start(out=dbg_h1[:], in_=h1T[:])

            # ---------------- Stage J: GAT aggregation
            adv = adst[:].rearrange("p (t h) -> p t h", h=2)
            oaccv = oacc[:].rearrange("p (t c) -> p t c", c=14)
            for (t0, T, Dg, base, Dg2, npass, base2) in groups:
                slot = io2.tile([P, SLOTCAP * 8], F32, tag="slot3", name="slot3", bufs=2)
                for c0 in range(0, T * Dg, GCHUNK):
                    cw = min(GCHUNK, T * Dg - c0)
                    gq(nc.gpsimd.indirect_dma_start(
                        out=slot[:, c0 * 8: (c0 + cw) * 8],
                        out_offset=None,
                        in_=table3.opt(),
                        in_offset=bass.IndirectOffsetOnAxis(
                            ap=idxt[:, base + c0: base + c0 + cw], axis=0
                        ),
                    ))
                sv = slot[:, : T * Dg * 8].bitcast(BF16).rearrange(
                    "p (t d f) -> p t d f", t=T, d=Dg
                )
                if DBG2:
                    nc.sync.dma_start(
                        out=dbg_s3[:, base * 8: (base + T * Dg) * 8],
                        in_=slot[:, : T * Dg * 8],
                    )
                for h in range(HEADS):
                    ev = io2.tile([P, SLOTCAP], F32, tag="ev")
                    e3 = ev[:, : T * Dg].rearrange("p (t d) -> p t d", t=T)
                    nc.vector.tensor_tensor(
                        out=e3, in0=sv[:, :, :, 14 + h],
                        in1=adv[:, t0:t0 + T, h][:, :, None].to_broadcast([P, T, Dg]),
                        op=AL.add,
                    )
                    # leaky relu 0.2 (device Lrelu alpha is broken -> manual)
                    et = io2.tile([P, SLOTCAP], F32, tag="et")
                    nc.vector.tensor_scalar(
                        out=et[:, : T * Dg], in0=ev[:, : T * Dg],
                        scalar1=0.2, scalar2=None, op0=AL.mult,
                    )
                    nc.vector.tensor_max(
                        out=ev[:, : T * Dg], in0=ev[:, : T * Dg], in1=et[:, : T * Dg]
                    )
                    if DBG2:
                        nc.sync.dma_start(
                            out=dbg_e[:, base * 2 + h * T * Dg: base * 2 + (h + 1) * T * Dg],
                            in_=ev[:, : T * Dg],
                        )
                    mx = io2.tile([P, MAXT], F32, tag="mx")
                    nc.vector.tensor_reduce(
                        out=mx[:, :T], in_=e3, axis=AX.X, op=AL.max
                    )
                    nc.vector.tensor_tensor(
                        out=e3, in0=e3,
                        in1=mx[:, :T, None].to_broadcast([P, T, Dg]),
                        op=AL.subtract,
                    )
                    pb = io2.tile([P, SLOTCAP], BF16, tag="pb")
                    nc.scalar.activation(
                        out=pb[:, : T * Dg], in_=ev[:, : T * Dg], func=AF.Exp
                    )
                    ss = io2.tile([P, MAXT], F32, tag="ss")
                    nc.vector.tensor_reduce(
                        out=ss[:, :T],
                        in_=pb[:, : T * Dg].rearrange("p (t d) -> p t d", t=T),
                        axis=AX.X, op=AL.add,
                    )
                    rc = io2.tile([P, MAXT], F32, tag="rc")
                    nc.vector.reciprocal(out=rc[:, :T], in_=ss[:, :T])
                    wb = io2.tile([P, SLOTCAP * CLS], BF16, tag="wb")
                    wv = wb[:, : T * Dg * CLS].rearrange(
                        "p (t d c) -> p t d c", t=T, d=Dg
                    )
                    nc.vector.tensor_tensor(
                        out=wv, in0=sv[:, :, :, h * CLS:(h + 1) * CLS],
                        in1=pb[:, : T * Dg].rearrange("p (t d) -> p t d", t=T)[
                            :, :, :, None
                        ].to_broadcast([P, T, Dg, CLS]),
                        op=AL.mult,
                    )
                    _tree_reduce(nc, wv, Dg)
                    nc.vector.tensor_tensor(
                        out=oaccv[:, t0:t0 + T, h * CLS:(h + 1) * CLS],
                        in0=wv[:, :, 0, :],
                        in1=rc[:, :T, None].to_broadcast([P, T, CLS]),
                        op=AL.mult,
                    )
            # + bg, log_softmax
            nc.vector.tensor_tensor(
                out=oaccv, in0=oaccv,
                in1=bgt[:][:, None, :].to_broadcast([P, NT, 14]), op=AL.add,
            )
            mx2 = cp.tile([P, NT], F32, tag="mx2")
            nc.vector.tensor_reduce(out=mx2[:], in_=oaccv, axis=AX.X, op=AL.max)
            nc.vector.tensor_tensor(
                out=oaccv, in0=oaccv,
                in1=mx2[:][:, :, None].to_broadcast([P, NT, 14]), op=AL.subtract,
            )
            ex = cp.tile([P, NT * 14], F32, tag="ex")
            nc.scalar.activation(out=ex[:], in_=oacc[:], func=AF.Exp)
            ssum = cp.tile([P, NT], F32, tag="ssum")
            nc.vector.tensor_reduce(
                out=ssum[:], in_=ex[:].rearrange("p (t c) -> p t c", c=14),
                axis=AX.X, op=AL.add,
            )
            lg = cp.tile([P, NT], F32, tag="lg")
            nc.scalar.activation(out=lg[:], in_=ssum[:], func=AF.Ln)
            nc.vector.tensor_tensor(
                out=oaccv, in0=oaccv,
                in1=lg[:][:, :, None].to_broadcast([P, NT, 14]), op=AL.subtract,
            )
            nc.sync.dma_start(
                out=out[:].rearrange("(rt p) c -> p rt c", p=P),
                in_=oaccv,
            )

    nc.compile()
    return nc


def kernel(x, edge_index, W1, b1, W2, b2, Wg, att_src, att_dst, bg, **_):
    x = np.asarray(x)
    edge_index = np.asarray(edge_index)
    W1 = np.asarray(W1, np.float32)
    b1 = np.asarray(b1, np.float32)
    W2 = np.asarray(W2, np.float32)
    b2 = np.asarray(b2, np.float32)
    Wg = np.asarray(Wg, np.float32)
    att_src = np.asarray(att_src, np.float32)
    att_dst = np.asarray(att_dst, np.float32)
    bg = np.asarray(bg, np.float32)

    perms, groups, S_gat, S_gcn, gidx, gidx2, dinv_arr = _preprocess(edge_index)

    key = (S_gat, S_gcn, tuple(groups))
    if key not in _cache:
        _cache[key] = _build(groups, S_gat, S_gcn)
    nc = _cache[key]

    # weights
    W1p = np.zeros((F_PAD, HID), np.float32)
    W1p[:F_IN] = W1
    w1d = np.ascontiguousarray(
        W1p.reshape(KC, P, HID).transpose(1, 0, 2).reshape(P, KC * HID)
    ).astype(BF)
    w2d = W2.astype(BF)
    wsrc = np.stack(
        [Wg[:, h * CLS:(h + 1) * CLS] @ att_src[h] for h in range(HEADS)], axis=1
    )
    wdst = np.stack(
        [Wg[:, h * CLS:(h + 1) * CLS] @ att_dst[h] for h in range(HEADS)], axis=1
    )
    wgd = np.concatenate([Wg, wsrc, wdst], axis=1).astype(BF)  # [64, 18]
    b1d = np.ascontiguousarray(np.broadcast_to(b1, (P, HID))).astype(BF)
    b2d = np.ascontiguousarray(np.broadcast_to(b2, (P, HID))).astype(BF)
    bgd = np.ascontiguousarray(np.broadcast_to(bg, (P, 14))).astype(np.float32)

    in_maps = []
    for k in range(NC):
        xk = x[k * NPC:(k + 1) * NPC][perms[k]]  # [NPC, F_IN]
        xTk = np.zeros((F_PAD, NRANKS), BF)
        xTk[:F_IN, :NPC] = xk.T.astype(BF)
        in_maps.append({
            "xT": xTk,
            "w1": w1d, "w2": w2d, "wg": wgd,
            "b1": b1d, "b2": b2d, "bg": bgd,
            "dinv": dinv_arr[k],
            "gidx": gidx[k],
            **({"gidx2": gidx2[k]} if KACC > 1 else {}),
        })

    res = run_bass_kernel_spmd(nc, in_maps, core_ids=list(range(NC)))

    outf = np.empty((N, 14), np.float32)
    for k in range(NC):
        ok = res.results[k]["out"][:NPC]  # rank order
        outf[k * NPC + perms[k]] = ok
    return outf


if __name__ == "__main__":
    rng = np.random.default_rng(0)
    x = rng.standard_normal((N, F_IN)).astype(np.float32)
    ei = rng.integers(0, N, (2, 1600000)).astype(np.int32)
    W1 = rng.standard_normal((F_IN, HID)).astype(np.float32) * 0.02
    W2 = rng.standard_normal((HID, HID)).astype(np.float32) * 0.1
    Wg = rng.standard_normal((HID, HEADS * CLS)).astype(np.float32) * 0.1
    o = kernel(
        x, ei, W1, np.zeros(HID, np.float32), W2, np.zeros(HID, np.float32),
        Wg, rng.standard_normal((HEADS, CLS)).astype(np.float32) * 0.1,
        rng.standard_normal((HEADS, CLS)).astype(np.float32) * 0.1,
        np.zeros(HEADS * CLS, np.float32),
    )
    print("kernel output", o.shape, o[:2])



# revision 20
# speedup vs baseline: 1.4066x; 1.4066x over previous
"""EnhancedGCN (2x GCNConv + GAT head + log_softmax) on 8 Trainium2 NeuronCores.

Strategy (graph/data parallel, per sharding hint):
- Nodes sharded contiguously across 8 cores (12500 each); within each core,
  nodes are permuted by descending in-degree so fixed-slot edge buffers pad well.
- Each layer: local matmul (features bf16) -> scale rows by dinv[src] -> chunked
  AllGather of the transformed node table (overlapped with surrounding compute)
  -> each core pull-gathers its in-edges' source rows with one indirect DMA per
  slot-group -> tree-reduce slots -> scale by dinv[dst], bias, relu. GAT head
  gathers [g(14) | a_src(2)] rows, does segment-softmax with fixed slots, then
  log_softmax.
- Tables use a chunk-interleaved row layout so each AllGather chunk is a
  contiguous slice: row(k, r) = NC*r0c + k*CHc + (r - r0c) for rank r in
  chunk [r0c, r1c), plus one shared zero/pad row at NC*NRANKS.
- All graph preprocessing (self-loops, degrees, slot layout, gather indices)
  is host-side numpy from edge_index only.
"""
import sys, os
sys.path.insert(0, '/opt/trn_rl_repo')

import numpy as np
import ml_dtypes

import concourse.bass as bass
import concourse.bacc as bacc
from concourse import mybir
import concourse.tile as tile
from concourse.masks import make_identity
from concourse.bass_utils import run_bass_kernel_spmd

BF = ml_dtypes.bfloat16
F32 = mybir.dt.float32
BF16 = mybir.dt.bfloat16
I32 = mybir.dt.int32
AL = mybir.AluOpType
AF = mybir.ActivationFunctionType
AX = mybir.AxisListType

N = 100000
NC = 8
NPC = N // NC            # 12500 nodes per core
P = 128
NT = 98                  # dst tiles per core (98*128 = 12544 ranks, 44 dummy)
NRANKS = NT * P          # 12544
TROWS = NRANKS + 1       # +1 zero/pad row per core shard
TABROWS = NC * TROWS     # AllGathered table rows
F_IN = 1433
F_PAD = 1536             # 12 * 128
KC = F_PAD // P          # 12 k-chunks
HID = 64
CLS = 7
HEADS = 2
GW = 16                  # gat row: g(14) + a_src(2)
SLOTCAP = 128            # max slots (Tg*Dg) per partition per gather group
MAXT = 12                # max tiles per group
GCHUNK = int(os.environ.get("KGCHUNK", "64"))  # idx columns per indirect DMA call
NSWQ = int(os.environ.get("KNSWQ", "4"))       # SWDGE queues to round-robin
KACC = int(os.environ.get("KKACC", "1"))       # gather accumulate passes (GCN layers)
NCHT = int(os.environ.get("KNCHT", "4"))       # target AllGather chunks per table

_cache = {}


def _make_groups(D_t):
    """groups: (t0, T, Dg, gat_base, Dg2, npass, gcn_base)."""
    groups = []
    t0 = 0
    gat_base = 0
    gcn_base = 0
    while t0 < NT:
        Dg = int(D_t[t0])
        Dg2 = (Dg + KACC - 1) // KACC
        npass = (Dg + Dg2 - 1) // Dg2
        T = 1
        while (t0 + T < NT) and (T < MAXT) and (Dg * (T + 1) <= SLOTCAP):
            T += 1
        groups.append((t0, T, Dg, gat_base, Dg2, npass, gcn_base))
        gat_base += T * Dg
        gcn_base += T * Dg2 * npass
        t0 += T
    return groups, gat_base, gcn_base


def _make_chunks(groups):
    """Split groups into ~NCHT chunks of roughly equal rank counts.

    Returns list of (g_lo, g_hi, r0, r1); every chunk is group-aligned and
    capped at 4096 ranks (PSUM pairing limit in stage A is 8 rowblocks)."""
    chunks = []
    g_lo = 0
    r0 = 0
    acc = 0
    target = NRANKS / NCHT
    for gi, g in enumerate(groups):
        gw = g[1] * P
        if acc > 0 and (acc + gw > 4096 or acc >= target):
            chunks.append((g_lo, gi, r0, r0 + acc))
            g_lo = gi
            r0 += acc
            acc = 0
        acc += gw
    chunks.append((g_lo, len(groups), r0, r0 + acc))
    assert r0 + acc == NRANKS
    return chunks


def _preprocess(edge_index):
    src0 = edge_index[0].astype(np.int64)
    dst0 = edge_index[1].astype(np.int64)
    loop = np.arange(N, dtype=np.int64)
    src = np.concatenate([src0, loop])
    dst = np.concatenate([dst0, loop])
    deg = np.bincount(dst, minlength=N)
    dinv = (1.0 / np.sqrt(np.maximum(deg, 1))).astype(np.float32)
    dinv[deg == 0] = 0.0

    perms = []
    rank_of = np.empty(N, np.int64)
    degs_sorted = np.empty((NC, NPC), np.int64)
    for k in range(NC):
        ld = deg[k * NPC:(k + 1) * NPC]
        perm = np.argsort(-ld, kind='stable')
        perms.append(perm)
        rank_of[k * NPC + perm] = np.arange(NPC)
        degs_sorted[k] = ld[perm]

    dpad = np.zeros((NC, NRANKS), np.int64)
    dpad[:, :NPC] = degs_sorted
    D_t = np.maximum(dpad[:, 0:NRANKS:P].max(axis=0), 1)  # [NT]
    groups, S_gat, S_gcn = _make_groups(D_t)
    chunks = _make_chunks(groups)

    # table row for node n: core(n) * TROWS + rank(n); pad row per core
    tpos = (np.arange(N) // NPC) * TROWS + rank_of
    owner = dst // NPC
    esp_all = tpos[src]
    gidx = np.empty((NC, P, S_gat), np.int32)
    gidx2 = np.empty((NC, P, S_gcn), np.int32) if KACC > 1 else None
    dinv_arr = np.zeros((NC, P, NT), np.float32)
    for k in range(NC):
        sel = owner == k
        esp = esp_all[sel]
        er = rank_of[dst[sel]]
        order = np.argsort(er, kind='stable')
        esp = esp[order]
        cnt = np.bincount(er[order], minlength=NRANKS)
        roff = np.concatenate([[0], np.cumsum(cnt)])
        zrow = k * TROWS + NRANKS
        idxk = np.full((P, S_gat), zrow, np.int32)
        idxk2 = np.full((P, S_gcn), zrow, np.int32) if KACC > 1 else None
        for (t0, T, Dg, base, Dg2, npass, base2) in groups:
            for tt in range(T):
                t = t0 + tt
                ranks = t * P + np.arange(P)
                c = cnt[ranks]
                sidx = roff[ranks][:, None] + np.arange(Dg)[None, :]
                mask = np.arange(Dg)[None, :] < c[:, None]
                vals = np.where(mask, esp[np.minimum(sidx, len(esp) - 1)], zrow)
                idxk[:, base + tt * Dg: base + (tt + 1) * Dg] = vals
                if KACC > 1:
                    # GCN layout: edge s -> pass s // Dg2, pos s % Dg2
                    for pp in range(npass):
                        lo = pp * Dg2
                        w = min(Dg2, Dg - lo)
                        dst_cols = base2 + pp * T * Dg2 + tt * Dg2
                        idxk2[:, dst_cols: dst_cols + w] = vals[:, lo: lo + w]
        gidx[k] = idxk
        if KACC > 1:
            gidx2[k] = idxk2
        dvp = np.zeros(NRANKS, np.float32)
        dvp[:NPC] = dinv[k * NPC:(k + 1) * NPC][perms[k]]
        dinv_arr[k] = dvp.reshape(NT, P).T
    return perms, groups, chunks, S_gat, S_gcn, gidx, gidx2, dinv_arr


def _tree_reduce(nc, view, Dg):
    """In-place pairwise sum over the slot axis of view [128, T, D, F*]."""
    dd = Dg
    while dd > 1:
        h = dd // 2
        nd = dd - h
        nc.vector.tensor_add(
            out=view[:, :, 0:h, :], in0=view[:, :, 0:h, :], in1=view[:, :, nd:dd, :]
        )
        dd = nd


def _build(groups, chunks, S_gat, S_gcn):
    nc = bacc.Bacc("TRN2", target_bir_lowering=False, debug=False, num_devices=NC,
                   num_swdge_queues=NSWQ)
    qrr = [0]

    def gq(inst):
        if NSWQ > 1:
            i = qrr[0] % NSWQ
            qrr[0] += 1
            if i:
                inst.ins.queue = f"qPoolDynamic{i}"
        return inst

    mw = max(r1 - r0 for (_, _, r0, r1) in chunks)  # max chunk rank width

    xT = nc.dram_tensor("xT", [F_PAD, NRANKS], BF16, kind="ExternalInput")
    w1 = nc.dram_tensor("w1", [P, KC * HID], BF16, kind="ExternalInput")
    w2 = nc.dram_tensor("w2", [HID, HID], BF16, kind="ExternalInput")
    wg = nc.dram_tensor("wg", [HID, 18], BF16, kind="ExternalInput")
    b1 = nc.dram_tensor("b1", [P, HID], BF16, kind="ExternalInput")
    b2 = nc.dram_tensor("b2", [P, HID], BF16, kind="ExternalInput")
    bg = nc.dram_tensor("bg", [P, 14], F32, kind="ExternalInput")
    dinv = nc.dram_tensor("dinv", [P, NT], F32, kind="ExternalInput")
    gidx = nc.dram_tensor("gidx", [P, S_gat], I32, kind="ExternalInput")
    gidx2 = (nc.dram_tensor("gidx2", [P, S_gcn], I32, kind="ExternalInput")
             if KACC > 1 else None)
    out = nc.dram_tensor("out", [P, NT * 14], F32, kind="ExternalOutput")

    with tile.TileContext(nc) as tc:
        with tc.tile_pool(name="const", bufs=1) as cp, \
             tc.tile_pool(name="io2", bufs=2) as io2, \
             tc.tile_pool(name="io3", bufs=3) as io3, \
             tc.tile_pool(name="slotp", bufs=3) as slotp, \
             tc.tile_pool(name="psmm", bufs=4, space="PSUM") as psmm, \
             tc.tile_pool(name="pstp", bufs=4, space="PSUM") as pstp, \
             tc.tile_pool(name="dram", bufs=1, space="DRAM") as dram:

            ident = cp.tile([P, P], F32, tag="ident")
            make_identity(nc, ident[:])
            w1t = cp.tile([P, KC * HID], BF16, tag="w1t")
            nc.sync.dma_start(w1t[:], w1[:])
            w2t = cp.tile([HID, HID], BF16, tag="w2t")
            nc.sync.dma_start(w2t[:], w2[:])
            wgt = cp.tile([HID, 18], BF16, tag="wgt")
            nc.sync.dma_start(wgt[:], wg[:])
            b1t = cp.tile([P, HID], BF16, tag="b1t")
            nc.sync.dma_start(b1t[:], b1[:])
            b2t = cp.tile([P, HID], BF16, tag="b2t")
            nc.sync.dma_start(b2t[:], b2[:])
            bgt = cp.tile([P, 14], F32, tag="bgt")
            nc.sync.dma_start(bgt[:], bg[:])
            dvt = cp.tile([P, NT], F32, tag="dvt")
            nc.sync.dma_start(dvt[:], dinv[:])
            idxt = cp.tile([P, S_gat], I32, tag="idxt")
            nc.sync.dma_start(idxt[:], gidx[:])
            if KACC > 1:
                idxt2 = cp.tile([P, S_gcn], I32, tag="idxt2")
                nc.sync.dma_start(idxt2[:], gidx2[:])
            else:
                idxt2 = idxt

            zt = cp.tile([1, 32], F32, tag="zt")
            nc.vector.memset(zt[:], 0.0)
            gpad = cp.tile([1, GW], BF16, tag="gpad")
            nc.vector.memset(gpad[:, 0:14], 0.0)
            nc.vector.memset(gpad[:, 14:GW], -1e30)

            shard1 = dram.tile([TROWS, 32], F32, tag="shard1")
            shard2 = dram.tile([TROWS, 32], F32, tag="shard2")
            shard3 = dram.tile([TROWS, 8], F32, tag="shard3")
            # Shared DRAM tiles may only have a single writing instruction, so
            # each table is filled by exactly one AllGather (pad rows travel in
            # the shard).
            table1 = dram.tile([TABROWS, 32], F32, tag="table1", addr_space="Shared")
            table2 = dram.tile([TABROWS, 32], F32, tag="table2", addr_space="Shared")
            table3 = dram.tile([TABROWS, 8], F32, tag="table3", addr_space="Shared")

            adst = cp.tile([P, NT * 2], F32, tag="adst")
            oacc = cp.tile([P, NT * 14], F32, tag="oacc")
            h1T = cp.tile([HID, NRANKS], BF16, tag="h1T")
            h2T = h1T  # layer-1 activations are fully consumed by mm2 before
            # agg2 writes the same columns; reuse saves 24.5KB/partition SBUF

            def ag_full(shard, table):
                nc.gpsimd.collective_compute(
                    "AllGather", AL.bypass, replica_groups=[list(range(NC))],
                    ins=[shard.opt()], outs=[table.opt()],
                )

            # ---------------- Stage A: h1pre.T = W1.T @ x.T, scale, shard1,
            # AllGather1 chunk by chunk (AG overlaps the next chunk's matmul).
            acc1 = cp.tile([P, NT * HID], BF16, tag="acc1")
            for (g_lo, g_hi, c0, c1) in chunks:
                w = c1 - c0
                nrb = (w + 511) // 512
                npair = (nrb + 1) // 2
                psums = []
                for k in range(KC):
                    xt = io3.tile([P, mw], BF16, tag="xt")
                    nc.sync.dma_start(xt[:, :w], xT[k * P:(k + 1) * P, c0:c0 + w])
                    for pr in range(npair):
                        if k == 0:
                            psums.append(
                                psmm.tile([P, 512], F32, tag="mm", name="psmm_a")
                            )
                        rb0 = pr * 2
                        rw0 = min(512, w - rb0 * 512)
                        # skip_group_check: the sim's psum zero-region tracker
                        # ignores base_partition, so the two tile_position
                        # quadrant groups false-positive as one region.
                        nc.tensor.matmul(
                            out=psums[pr][0:HID, :rw0],
                            lhsT=w1t[:, k * HID:(k + 1) * HID],
                            rhs=xt[:, rb0 * 512: rb0 * 512 + rw0],
                            start=(k == 0), stop=(k == KC - 1),
                            tile_position=(0, 0), skip_group_check=True,
                        )
                        rb1 = pr * 2 + 1
                        if rb1 < nrb:
                            rw1 = min(512, w - rb1 * 512)
                            nc.tensor.matmul(
                                out=psums[pr][HID:2 * HID, :rw1],
                                lhsT=w1t[:, k * HID:(k + 1) * HID],
                                rhs=xt[:, rb1 * 512: rb1 * 512 + rw1],
                                start=(k == 0), stop=(k == KC - 1),
                                tile_position=(0, HID), skip_group_check=True,
                            )
                for pr in range(npair):
                    for half in range(2):
                        rb = pr * 2 + half
                        if rb >= nrb:
                            continue
                        rw = min(512, w - rb * 512)
                        stg = io2.tile([HID, 512], F32, tag="stg")
                        nc.vector.tensor_copy(
                            out=stg[:, :rw],
                            in_=psums[pr][half * HID:(half + 1) * HID, :rw],
                        )
                        for b in range(rw // P):
                            rt = (c0 + rb * 512 + b * P) // P
                            tp = pstp.tile([P, HID], F32, tag="tp")
                            nc.tensor.transpose(
                                out=tp[:], in_=stg[:, b * P:(b + 1) * P],
                                identity=ident[0:HID, 0:HID],
                            )
                            nc.vector.tensor_scalar(
                                out=acc1[:, rt * HID:(rt + 1) * HID],
                                in0=tp[:], scalar1=dvt[:, rt:rt + 1], scalar2=None,
                                op0=AL.mult,
                            )
                nc.sync.dma_start(
                    out=shard1[c0:c1, :].rearrange("(rt p) c -> p rt c", p=P),
                    in_=acc1[:, c0 // P * HID:c1 // P * HID].bitcast(F32)
                        .rearrange("p (rt c) -> p rt c", rt=(c1 - c0) // P),
                )
            nc.sync.dma_start(out=shard1[NRANKS:TROWS, :], in_=zt[:])
            ag_full(shard1, table1)

            # ---------------- GCN aggregation + next-layer matmul, pipelined.
            # after_chunk(r0, r1) runs the dependent matmul + shard write for
            # the finished rank range so it overlaps the next chunk's gathers;
            # the (single) AllGather is emitted by the caller afterwards.
            def gcn_aggregate(table, btile, hT, after_chunk):
                for (g_lo, g_hi, r0, r1) in chunks:
                    for gi in range(g_lo, g_hi):
                        (t0, T, Dg, base, Dg2, npass, base2) = groups[gi]
                        slot = slotp.tile([P, SLOTCAP * 32], F32, tag="slot")
                        for pp in range(npass):
                            op = AL.bypass if pp == 0 else AL.add
                            pbase = base2 + pp * T * Dg2 if KACC > 1 else base
                            for cc0 in range(0, T * Dg2, GCHUNK):
                                cw = min(GCHUNK, T * Dg2 - cc0)
                                gq(nc.gpsimd.indirect_dma_start(
                                    out=slot[:, cc0 * 32: (cc0 + cw) * 32],
                                    out_offset=None,
                                    in_=table.opt(),
                                    in_offset=bass.IndirectOffsetOnAxis(
                                        ap=idxt2[:, pbase + cc0: pbase + cc0 + cw],
                                        axis=0
                                    ),
                                    compute_op=op,
                                ))
                        sv = slot[:, : T * Dg2 * 32].bitcast(BF16).rearrange(
                            "p (t d f) -> p t d f", t=T, d=Dg2
                        )
                        _tree_reduce(nc, sv, Dg2)
                        hbuf = io2.tile([P, MAXT * HID], F32, tag="hbuf")
                        for tt in range(T):
                            nc.vector.scalar_tensor_tensor(
                                out=hbuf[:, tt * HID:(tt + 1) * HID],
                                in0=sv[:, tt, 0, :],
                                scalar=dvt[:, t0 + tt: t0 + tt + 1],
                                in1=btile[:],
                                op0=AL.mult, op1=AL.add,
                            )
                        nc.scalar.activation(
                            out=hbuf[:, : T * HID], in_=hbuf[:, : T * HID],
                            func=AF.Relu
                        )
                        for tt in range(T):
                            tph = pstp.tile([HID, P], F32, tag="tp")
                            nc.tensor.transpose(
                                out=tph[:], in_=hbuf[:, tt * HID:(tt + 1) * HID],
                                identity=ident[:],
                            )
                            # Relu == exact copy here (hbuf is post-relu, >=0)
                            # and its ACT path is already exercised on HW.
                            nc.scalar.activation(
                                out=hT[:, (t0 + tt) * P:(t0 + tt + 1) * P],
                                in_=tph[:], func=AF.Relu,
                            )
                    after_chunk(r0, r1)

            # ---------------- L2 matmul on a finished h1T rank range
            def mm2_chunk(r0, r1):
                for c0 in range(r0, r1, 512):
                    w = min(512, r1 - c0)
                    ps2 = psmm.tile([HID, 512], F32, tag="mm")
                    nc.tensor.matmul(
                        out=ps2[:, :w], lhsT=w2t[:], rhs=h1T[:, c0:c0 + w],
                        start=True, stop=True,
                    )
                    stg = io2.tile([HID, 512], F32, tag="stg")
                    nc.vector.tensor_copy(out=stg[:, :w], in_=ps2[:, :w])
                    for b in range(w // P):
                        rt = (c0 + b * P) // P
                        tp = pstp.tile([P, HID], F32, tag="tp")
                        nc.tensor.transpose(
                            out=tp[:], in_=stg[:, b * P:(b + 1) * P],
                            identity=ident[0:HID, 0:HID],
                        )
                        nc.vector.tensor_scalar(
                            out=acc2[:, rt * HID:(rt + 1) * HID],
                            in0=tp[:], scalar1=dvt[:, rt:rt + 1], scalar2=None,
                            op0=AL.mult,
                        )
                nc.sync.dma_start(
                    out=shard2[r0:r1, :].rearrange("(rt p) c -> p rt c", p=P),
                    in_=acc2[:, r0 // P * HID:r1 // P * HID].bitcast(F32)
                        .rearrange("p (rt c) -> p rt c", rt=(r1 - r0) // P),
                )

            # ---------------- GAT node transform on a finished h2T rank range
            def mm3_chunk(r0, r1):
                for c0 in range(r0, r1, 512):
                    w = min(512, r1 - c0)
                    ps3 = psmm.tile([18, 512], F32, tag="mm")
                    nc.tensor.matmul(
                        out=ps3[:, :w], lhsT=wgt[:], rhs=h2T[:, c0:c0 + w],
                        start=True, stop=True,
                    )
                    stg = io2.tile([18, 512], F32, tag="stg3")
                    nc.vector.tensor_copy(out=stg[:, :w], in_=ps3[:, :w])
                    for b in range(w // P):
                        rt = (c0 + b * P) // P
                        tp = pstp.tile([P, 18], F32, tag="tp")
                        nc.tensor.transpose(
                            out=tp[:], in_=stg[:, b * P:(b + 1) * P],
                            identity=ident[0:18, 0:18],
                        )
                        nc.vector.tensor_copy(
                            out=acc3[:, rt * GW: rt * GW + GW], in_=tp[:, 0:GW]
                        )
                        nc.vector.tensor_copy(
                            out=adst[:, rt * 2: rt * 2 + 2], in_=tp[:, 16:18]
                        )
                nc.sync.dma_start(
                    out=shard3[r0:r1, :].rearrange("(rt p) c -> p rt c", p=P),
                    in_=acc3[:, r0 // P * GW:r1 // P * GW].bitcast(F32)
                        .rearrange("p (rt c) -> p rt c", rt=(r1 - r0) // P),
                )

            acc2 = cp.tile([P, NT * HID], BF16, tag="acc2")
            acc3 = cp.tile([P, NT * GW], BF16, tag="acc3")

            # ---------------- L1 aggregation -> h1T (mm2 pipelined in)
            gcn_aggregate(table1, b1t, h1T, mm2_chunk)
            nc.sync.dma_start(out=shard2[NRANKS:TROWS, :], in_=zt[:])
            ag_full(shard2, table2)

            # ---------------- L2 aggregation -> h2T (mm3 pipelined in)
            gcn_aggregate(table2, b2t, h2T, mm3_chunk)
            nc.sync.dma_start(out=shard3[NRANKS:TROWS, :], in_=gpad[:].bitcast(F32))
            ag_full(shard3, table3)

            # ---------------- Stage J: GAT aggregation (heads fused)
            adv = adst[:].rearrange("p (t h) -> p t h", h=2)
            oaccv = oacc[:].rearrange("p (t c) -> p t c", c=14)
            for (t0, T, Dg, base, Dg2, npass, base2) in groups:
                slot = slotp.tile([P, SLOTCAP * 8], F32, tag="slot3", name="slot3")
                for cc0 in range(0, T * Dg, GCHUNK):
                    cw = min(GCHUNK, T * Dg - cc0)
                    gq(nc.gpsimd.indirect_dma_start(
                        out=slot[:, cc0 * 8: (cc0 + cw) * 8],
                        out_offset=None,
                        in_=table3.opt(),
                        in_offset=bass.IndirectOffsetOnAxis(
                            ap=idxt[:, base + cc0: base + cc0 + cw], axis=0
                        ),
                    ))
                sv = slot[:, : T * Dg * 8].bitcast(BF16).rearrange(
                    "p (t d f) -> p t d f", t=T, d=Dg
                )
                TDg = T * Dg
                # e = a_src[src] + a_dst[dst], head-major flat: head h at
                # cols [h*TDg, (h+1)*TDg), each viewed [P, T, Dg]
                ev = io2.tile([P, SLOTCAP * 2], F32, tag="ev")
                for h in range(HEADS):
                    nc.vector.tensor_tensor(
                        out=ev[:, h * TDg:(h + 1) * TDg].rearrange(
                            "p (t d) -> p t d", t=T),
                        in0=sv[:, :, :, 14 + h],
                        in1=adv[:, t0:t0 + T, h][:, :, None].to_broadcast(
                            [P, T, Dg]),
                        op=AL.add,
                    )
                # leaky relu 0.2 both heads at once (device Lrelu is broken)
                et = io2.tile([P, SLOTCAP * 2], F32, tag="et")
                nc.vector.tensor_scalar(
                    out=et[:, : 2 * TDg], in0=ev[:, : 2 * TDg],
                    scalar1=0.2, scalar2=None, op0=AL.mult,
                )
                nc.vector.tensor_max(
                    out=ev[:, : 2 * TDg], in0=ev[:, : 2 * TDg],
                    in1=et[:, : 2 * TDg]
                )
                # segment max per (head, tile), subtract (baseline 3-D forms)
                mx = io2.tile([P, MAXT * 2], F32, tag="mx")
                for h in range(HEADS):
                    e3 = ev[:, h * TDg:(h + 1) * TDg].rearrange(
                        "p (t d) -> p t d", t=T)
                    nc.vector.tensor_reduce(
                        out=mx[:, h * T:(h + 1) * T], in_=e3, axis=AX.X,
                        op=AL.max,
                    )
                    nc.vector.tensor_tensor(
                        out=e3, in0=e3,
                        in1=mx[:, h * T:(h + 1) * T][:, :, None].to_broadcast(
                            [P, T, Dg]),
                        op=AL.subtract,
                    )
                pb = io2.tile([P, SLOTCAP * 2], BF16, tag="pb")
                nc.scalar.activation(
                    out=pb[:, : 2 * TDg], in_=ev[:, : 2 * TDg], func=AF.Exp
                )
                ss = io2.tile([P, MAXT * 2], F32, tag="ss")
                for h in range(HEADS):
                    nc.vector.tensor_reduce(
                        out=ss[:, h * T:(h + 1) * T],
                        in_=pb[:, h * TDg:(h + 1) * TDg].rearrange(
                            "p (t d) -> p t d", t=T),
                        axis=AX.X, op=AL.add,
                    )
                rc = io2.tile([P, MAXT * 2], F32, tag="rc")
                nc.vector.reciprocal(out=rc[:, : 2 * T], in_=ss[:, : 2 * T])
                # replicate alpha to 14 wide -> single weighted tree-reduce
                p14 = io2.tile([P, SLOTCAP * 14], BF16, tag="p14")
                p14v = p14[:, : TDg * 14].rearrange(
                    "p (t d c) -> p t d c", t=T, d=Dg)
                for h in range(HEADS):
                    nc.vector.tensor_copy(
                        out=p14v[:, :, :, h * CLS:(h + 1) * CLS],
                        in_=pb[:, h * TDg:(h + 1) * TDg].rearrange(
                            "p (t d) -> p t d", t=T)[:, :, :, None].to_broadcast(
                            [P, T, Dg, CLS]),
                    )
                wb = io2.tile([P, SLOTCAP * 14], BF16, tag="wb")
                wv = wb[:, : TDg * 14].rearrange(
                    "p (t d c) -> p t d c", t=T, d=Dg)
                nc.vector.tensor_tensor(
                    out=wv, in0=sv[:, :, :, 0:14],
                    in1=p14v, op=AL.mult,
                )
                _tree_reduce(nc, wv, Dg)
                for h in range(HEADS):
                    nc.vector.tensor_tensor(
                        out=oaccv[:, t0:t0 + T, h * CLS:(h + 1) * CLS],
                        in0=wv[:, :, 0, h * CLS:(h + 1) * CLS],
                        in1=rc[:, h * T:(h + 1) * T][:, :, None].to_broadcast(
                            [P, T, CLS]),
                        op=AL.mult,
                    )
            # + bg, log_softmax
            nc.vector.tensor_tensor(
                out=oaccv, in0=oaccv,
                in1=bgt[:][:, None, :].to_broadcast([P, NT, 14]), op=AL.add,
            )
            mx2 = cp.tile([P, NT], F32, tag="mx2")
            nc.vector.tensor_reduce(out=mx2[:], in_=oaccv, axis=AX.X, op=AL.max)
            nc.vector.tensor_tensor(
                out=oaccv, in0=oaccv,
                in1=mx2[:][:, :, None].to_broadcast([P, NT, 14]), op=AL.subtract,
            )
            ex = cp.tile([P, NT * 14], F32, tag="ex")
            nc.scalar.activation(out=ex[:], in_=oacc[:], func=AF.Exp)
            ssum = cp.tile([P, NT], F32, tag="ssum")
            nc.vector.tensor_reduce(
                out=ssum[:], in_=ex[:].rearrange("p (t c) -> p t c", c=14),
                axis=AX.X, op=AL.add,
            )
            lg = cp.tile([P, NT], F32, tag="lg")
            nc.scalar.activation(out=lg[:], in_=ssum[:], func=AF.Ln)
            nc.vector.tensor_tensor(
                out=oaccv, in0=oaccv,
                in1=lg[:][:, :, None].to_broadcast([P, NT, 14]), op=AL.subtract,
            )
            nc.sync.dma_start(out=out[:], in_=oacc[:])

    nc.compile()
    return nc


def _prepare(x, edge_index, W1, b1, W2, b2, Wg, att_src, att_dst, bg):
    """Preprocess + build/caching + per-core input maps. Shared with test.py."""
    perms, groups, chunks, S_gat, S_gcn, gidx, gidx2, dinv_arr = \
        _preprocess(edge_index)

    key = (S_gat, S_gcn, tuple(groups), tuple(chunks))
    if key not in _cache:
        _cache[key] = _build(groups, chunks, S_gat, S_gcn)
    nc = _cache[key]

    W1p = np.zeros((F_PAD, HID), np.float32)
    W1p[:F_IN] = W1
    w1d = np.ascontiguousarray(
        W1p.reshape(KC, P, HID).transpose(1, 0, 2).reshape(P, KC * HID)
    ).astype(BF)
    w2d = W2.astype(BF)
    wsrc = np.stack(
        [Wg[:, h * CLS:(h + 1) * CLS] @ att_src[h] for h in range(HEADS)], axis=1
    )
    wdst = np.stack(
        [Wg[:, h * CLS:(h + 1) * CLS] @ att_dst[h] for h in range(HEADS)], axis=1
    )
    wgd = np.concatenate([Wg, wsrc, wdst], axis=1).astype(BF)  # [64, 18]
    b1d = np.ascontiguousarray(np.broadcast_to(b1, (P, HID))).astype(BF)
    b2d = np.ascontiguousarray(np.broadcast_to(b2, (P, HID))).astype(BF)
    bgd = np.ascontiguousarray(np.broadcast_to(bg, (P, 14))).astype(np.float32)

    in_maps = []
    for k in range(NC):
        xk = x[k * NPC:(k + 1) * NPC][perms[k]]  # [NPC, F_IN]
        xTk = np.zeros((F_PAD, NRANKS), BF)
        xTk[:F_IN, :NPC] = xk.T.astype(BF)
        in_maps.append({
            "xT": xTk,
            "w1": w1d, "w2": w2d, "wg": wgd,
            "b1": b1d, "b2": b2d, "bg": bgd,
            "dinv": dinv_arr[k],
            "gidx": gidx[k],
            **({"gidx2": gidx2[k]} if KACC > 1 else {}),
        })
    return nc, in_maps, perms


def _unshard(res, perms):
    outf = np.empty((N, 14), np.float32)
    for k in range(NC):
        ok = res.results[k]["out"].reshape(P, NT, 14).transpose(1, 0, 2) \
            .reshape(NRANKS, 14)[:NPC]  # rank order
        outf[k * NPC + perms[k]] = ok
    return outf


def kernel(x, edge_index, W1, b1, W2, b2, Wg, att_src, att_dst, bg, **_):
    x = np.asarray(x)
    edge_index = np.asarray(edge_index)
    W1 = np.asarray(W1, np.float32)
    b1 = np.asarray(b1, np.float32)
    W2 = np.asarray(W2, np.float32)
    b2 = np.asarray(b2, np.float32)
    Wg = np.asarray(Wg, np.float32)
    att_src = np.asarray(att_src, np.float32)
    att_dst = np.asarray(att_dst, np.float32)
    bg = np.asarray(bg, np.float32)

    nc, in_maps, perms = _prepare(x, edge_index, W1, b1, W2, b2, Wg,
                                  att_src, att_dst, bg)
    res = run_bass_kernel_spmd(nc, in_maps, core_ids=list(range(NC)))
    return _unshard(res, perms)


if __name__ == "__main__":
    rng = np.random.default_rng(0)
    x = rng.standard_normal((N, F_IN)).astype(np.float32)
    ei = rng.integers(0, N, (2, 1600000)).astype(np.int32)
    W1 = rng.standard_normal((F_IN, HID)).astype(np.float32) * 0.02
    W2 = rng.standard_normal((HID, HID)).astype(np.float32) * 0.1
    Wg = rng.standard_normal((HID, HEADS * CLS)).astype(np.float32) * 0.1
    o = kernel(
        x, ei, W1, np.zeros(HID, np.float32), W2, np.zeros(HID, np.float32),
        Wg, rng.standard_normal((HEADS, CLS)).astype(np.float32) * 0.1,
        rng.standard_normal((HEADS, CLS)).astype(np.float32) * 0.1,
        np.zeros(HEADS * CLS, np.float32),
    )
    print("kernel output", o.shape, o[:2])


# revision 21
# speedup vs baseline: 1.5332x; 1.0900x over previous
"""EnhancedGCN (2x GCNConv + GAT head + log_softmax) on 8 Trainium2 NeuronCores.

Strategy (graph/data parallel, per sharding hint):
- Nodes sharded contiguously across 8 cores (12500 each); within each core,
  nodes are permuted by descending in-degree so fixed-slot edge buffers pad well.
- Each layer: local matmul (features bf16) -> scale rows by dinv[src] -> chunked
  AllGather of the transformed node table (overlapped with surrounding compute)
  -> each core pull-gathers its in-edges' source rows with one indirect DMA per
  slot-group -> tree-reduce slots -> scale by dinv[dst], bias, relu. GAT head
  gathers [g(14) | a_src(2)] rows, does segment-softmax with fixed slots, then
  log_softmax.
- Tables use a chunk-interleaved row layout so each AllGather chunk is a
  contiguous slice: row(k, r) = NC*r0c + k*CHc + (r - r0c) for rank r in
  chunk [r0c, r1c), plus one shared zero/pad row at NC*NRANKS.
- All graph preprocessing (self-loops, degrees, slot layout, gather indices)
  is host-side numpy from edge_index only.
"""
import sys, os
sys.path.insert(0, '/opt/trn_rl_repo')

import numpy as np
import ml_dtypes

import concourse.bass as bass
import concourse.bacc as bacc
from concourse import mybir
import concourse.tile as tile
from concourse.masks import make_identity
from concourse.bass_utils import run_bass_kernel_spmd

BF = ml_dtypes.bfloat16
F8 = ml_dtypes.float8_e4m3
F32 = mybir.dt.float32
BF16 = mybir.dt.bfloat16
F8D = mybir.dt.float8e4
I32 = mybir.dt.int32
AL = mybir.AluOpType
AF = mybir.ActivationFunctionType
AX = mybir.AxisListType

N = 100000
NC = 8
NPC = N // NC            # 12500 nodes per core
P = 128
NT = 98                  # dst tiles per core (98*128 = 12544 ranks, 44 dummy)
NRANKS = NT * P          # 12544
TROWS = NRANKS + 1       # +1 zero/pad row per core shard
TABROWS = NC * TROWS     # AllGathered table rows
F_IN = 1433
F_PAD = 1536             # 12 * 128
KC = F_PAD // P          # 12 k-chunks
HID = 64
CLS = 7
HEADS = 2
GW = 16                  # gat row: g(14) + a_src(2)
SLOTCAP = 128            # max slots (Tg*Dg) per partition per gather group
MAXT = 12                # max tiles per group
GCHUNK = int(os.environ.get("KGCHUNK", "64"))  # idx columns per indirect DMA call
NSWQ = int(os.environ.get("KNSWQ", "4"))       # SWDGE queues to round-robin
KACC = int(os.environ.get("KKACC", "1"))       # gather accumulate passes (GCN layers)
NCHT = int(os.environ.get("KNCHT", "4"))       # target AllGather chunks per table

_cache = {}


def _make_groups(D_t):
    """groups: (t0, T, Dg, gat_base, Dg2, npass, gcn_base)."""
    groups = []
    t0 = 0
    gat_base = 0
    gcn_base = 0
    while t0 < NT:
        Dg = int(D_t[t0])
        Dg2 = (Dg + KACC - 1) // KACC
        npass = (Dg + Dg2 - 1) // Dg2
        T = 1
        while (t0 + T < NT) and (T < MAXT) and (Dg * (T + 1) <= SLOTCAP):
            T += 1
        groups.append((t0, T, Dg, gat_base, Dg2, npass, gcn_base))
        gat_base += T * Dg
        gcn_base += T * Dg2 * npass
        t0 += T
    return groups, gat_base, gcn_base


def _make_chunks(groups):
    """Split groups into ~NCHT chunks of roughly equal rank counts.

    Returns list of (g_lo, g_hi, r0, r1); every chunk is group-aligned and
    capped at 4096 ranks (PSUM pairing limit in stage A is 8 rowblocks)."""
    chunks = []
    g_lo = 0
    r0 = 0
    acc = 0
    target = NRANKS / NCHT
    for gi, g in enumerate(groups):
        gw = g[1] * P
        if acc > 0 and (acc + gw > 4096 or acc >= target):
            chunks.append((g_lo, gi, r0, r0 + acc))
            g_lo = gi
            r0 += acc
            acc = 0
        acc += gw
    chunks.append((g_lo, len(groups), r0, r0 + acc))
    assert r0 + acc == NRANKS
    return chunks


def _preprocess(edge_index):
    src0 = edge_index[0].astype(np.int64)
    dst0 = edge_index[1].astype(np.int64)
    loop = np.arange(N, dtype=np.int64)
    src = np.concatenate([src0, loop])
    dst = np.concatenate([dst0, loop])
    deg = np.bincount(dst, minlength=N)
    dinv = (1.0 / np.sqrt(np.maximum(deg, 1))).astype(np.float32)
    dinv[deg == 0] = 0.0

    perms = []
    rank_of = np.empty(N, np.int64)
    degs_sorted = np.empty((NC, NPC), np.int64)
    for k in range(NC):
        ld = deg[k * NPC:(k + 1) * NPC]
        perm = np.argsort(-ld, kind='stable')
        perms.append(perm)
        rank_of[k * NPC + perm] = np.arange(NPC)
        degs_sorted[k] = ld[perm]

    dpad = np.zeros((NC, NRANKS), np.int64)
    dpad[:, :NPC] = degs_sorted
    D_t = np.maximum(dpad[:, 0:NRANKS:P].max(axis=0), 1)  # [NT]
    groups, S_gat, S_gcn = _make_groups(D_t)
    chunks = _make_chunks(groups)

    # table row for node n: core(n) * TROWS + rank(n); pad row per core
    tpos = (np.arange(N) // NPC) * TROWS + rank_of
    owner = dst // NPC
    esp_all = tpos[src]
    gidx = np.empty((NC, P, S_gat), np.int32)
    gidx2 = np.empty((NC, P, S_gcn), np.int32) if KACC > 1 else None
    dinv_arr = np.zeros((NC, P, NT), np.float32)
    for k in range(NC):
        sel = owner == k
        esp = esp_all[sel]
        er = rank_of[dst[sel]]
        order = np.argsort(er, kind='stable')
        esp = esp[order]
        cnt = np.bincount(er[order], minlength=NRANKS)
        roff = np.concatenate([[0], np.cumsum(cnt)])
        zrow = k * TROWS + NRANKS
        idxk = np.full((P, S_gat), zrow, np.int32)
        idxk2 = np.full((P, S_gcn), zrow, np.int32) if KACC > 1 else None
        for (t0, T, Dg, base, Dg2, npass, base2) in groups:
            for tt in range(T):
                t = t0 + tt
                ranks = t * P + np.arange(P)
                c = cnt[ranks]
                sidx = roff[ranks][:, None] + np.arange(Dg)[None, :]
                mask = np.arange(Dg)[None, :] < c[:, None]
                vals = np.where(mask, esp[np.minimum(sidx, len(esp) - 1)], zrow)
                idxk[:, base + tt * Dg: base + (tt + 1) * Dg] = vals
                if KACC > 1:
                    # GCN layout: edge s -> pass s // Dg2, pos s % Dg2
                    for pp in range(npass):
                        lo = pp * Dg2
                        w = min(Dg2, Dg - lo)
                        dst_cols = base2 + pp * T * Dg2 + tt * Dg2
                        idxk2[:, dst_cols: dst_cols + w] = vals[:, lo: lo + w]
        gidx[k] = idxk
        if KACC > 1:
            gidx2[k] = idxk2
        dvp = np.zeros(NRANKS, np.float32)
        dvp[:NPC] = dinv[k * NPC:(k + 1) * NPC][perms[k]]
        dinv_arr[k] = dvp.reshape(NT, P).T
    return perms, groups, chunks, S_gat, S_gcn, gidx, gidx2, dinv_arr


def _tree_reduce(nc, view, Dg):
    """In-place pairwise sum over the slot axis of view [128, T, D, F*]."""
    dd = Dg
    while dd > 1:
        h = dd // 2
        nd = dd - h
        nc.vector.tensor_add(
            out=view[:, :, 0:h, :], in0=view[:, :, 0:h, :], in1=view[:, :, nd:dd, :]
        )
        dd = nd


def _build(groups, chunks, S_gat, S_gcn):
    nc = bacc.Bacc("TRN2", target_bir_lowering=False, debug=False, num_devices=NC,
                   num_swdge_queues=NSWQ)
    qrr = [0]

    def gq(inst):
        if NSWQ > 1:
            i = qrr[0] % NSWQ
            qrr[0] += 1
            if i:
                inst.ins.queue = f"qPoolDynamic{i}"
        return inst

    mw = max(r1 - r0 for (_, _, r0, r1) in chunks)  # max chunk rank width

    xT = nc.dram_tensor("xT", [F_PAD, NRANKS], F8D, kind="ExternalInput")
    w1 = nc.dram_tensor("w1", [P, KC * HID], F8D, kind="ExternalInput")
    w2 = nc.dram_tensor("w2", [HID, HID], BF16, kind="ExternalInput")
    wg = nc.dram_tensor("wg", [HID, 18], BF16, kind="ExternalInput")
    b1 = nc.dram_tensor("b1", [P, HID], BF16, kind="ExternalInput")
    b2 = nc.dram_tensor("b2", [P, HID], BF16, kind="ExternalInput")
    bg = nc.dram_tensor("bg", [P, 14], F32, kind="ExternalInput")
    dinv = nc.dram_tensor("dinv", [P, NT], F32, kind="ExternalInput")
    gidx = nc.dram_tensor("gidx", [P, S_gat], I32, kind="ExternalInput")
    gidx2 = (nc.dram_tensor("gidx2", [P, S_gcn], I32, kind="ExternalInput")
             if KACC > 1 else None)
    out = nc.dram_tensor("out", [P, NT * 14], F32, kind="ExternalOutput")

    with tile.TileContext(nc) as tc:
        with tc.tile_pool(name="const", bufs=1) as cp, \
             tc.tile_pool(name="io2", bufs=2) as io2, \
             tc.tile_pool(name="io3", bufs=3) as io3, \
             tc.tile_pool(name="slotp", bufs=3) as slotp, \
             tc.tile_pool(name="psmm", bufs=4, space="PSUM") as psmm, \
             tc.tile_pool(name="pstp", bufs=4, space="PSUM") as pstp, \
             tc.tile_pool(name="dram", bufs=1, space="DRAM") as dram:

            ident = cp.tile([P, P], F32, tag="ident")
            make_identity(nc, ident[:])
            w1t = cp.tile([P, KC * HID], F8D, tag="w1t")
            nc.sync.dma_start(w1t[:], w1[:])
            w2t = cp.tile([HID, HID], BF16, tag="w2t")
            nc.sync.dma_start(w2t[:], w2[:])
            wgt = cp.tile([HID, 18], BF16, tag="wgt")
            nc.sync.dma_start(wgt[:], wg[:])
            b1t = cp.tile([P, HID], BF16, tag="b1t")
            nc.sync.dma_start(b1t[:], b1[:])
            b2t = cp.tile([P, HID], BF16, tag="b2t")
            nc.sync.dma_start(b2t[:], b2[:])
            bgt = cp.tile([P, 14], F32, tag="bgt")
            nc.sync.dma_start(bgt[:], bg[:])
            dvt = cp.tile([P, NT], F32, tag="dvt")
            nc.sync.dma_start(dvt[:], dinv[:])
            idxt = cp.tile([P, S_gat], I32, tag="idxt")
            nc.sync.dma_start(idxt[:], gidx[:])
            if KACC > 1:
                idxt2 = cp.tile([P, S_gcn], I32, tag="idxt2")
                nc.sync.dma_start(idxt2[:], gidx2[:])
            else:
                idxt2 = idxt

            zt = cp.tile([1, 16], F32, tag="zt")
            nc.vector.memset(zt[:], 0.0)
            gpad = cp.tile([1, GW], F8D, tag="gpad")
            nc.vector.memset(gpad[:, 0:14], 0.0)
            nc.vector.memset(gpad[:, 14:GW], -240.0)

            shard1 = dram.tile([TROWS, 16], F32, tag="shard1")
            shard2 = dram.tile([TROWS, 16], F32, tag="shard2")
            shard3 = dram.tile([TROWS, 4], F32, tag="shard3")
            # Shared DRAM tiles may only have a single writing instruction, so
            # each table is filled by exactly one AllGather (pad rows travel in
            # the shard).
            table1 = dram.tile([TABROWS, 16], F32, tag="table1", addr_space="Shared")
            table2 = dram.tile([TABROWS, 16], F32, tag="table2", addr_space="Shared")
            table3 = dram.tile([TABROWS, 4], F32, tag="table3", addr_space="Shared")

            adst = cp.tile([P, NT * 2], F32, tag="adst")
            oacc = cp.tile([P, NT * 14], F32, tag="oacc")
            h1T = cp.tile([HID, NRANKS], BF16, tag="h1T")
            h2T = h1T  # layer-1 activations are fully consumed by mm2 before
            # agg2 writes the same columns; reuse saves 24.5KB/partition SBUF

            def ag_full(shard, table):
                nc.gpsimd.collective_compute(
                    "AllGather", AL.bypass, replica_groups=[list(range(NC))],
                    ins=[shard.opt()], outs=[table.opt()],
                )

            # ---------------- Stage A: h1pre.T = W1.T @ x.T, scale, shard1,
            # AllGather1 chunk by chunk (AG overlaps the next chunk's matmul).
            acc1 = cp.tile([P, NT * HID], F8D, tag="acc1")
            for (g_lo, g_hi, c0, c1) in chunks:
                w = c1 - c0
                nrb = (w + 511) // 512
                npair = (nrb + 1) // 2
                psums = []
                for k in range(KC):
                    xt = io3.tile([P, mw], F8D, tag="xt")
                    nc.sync.dma_start(xt[:, :w], xT[k * P:(k + 1) * P, c0:c0 + w])
                    for pr in range(npair):
                        if k == 0:
                            psums.append(
                                psmm.tile([P, 512], F32, tag="mm", name="psmm_a")
                            )
                        rb0 = pr * 2
                        rw0 = min(512, w - rb0 * 512)
                        # skip_group_check: the sim's psum zero-region tracker
                        # ignores base_partition, so the two tile_position
                        # quadrant groups false-positive as one region.
                        nc.tensor.matmul(
                            out=psums[pr][0:HID, :rw0],
                            lhsT=w1t[:, k * HID:(k + 1) * HID],
                            rhs=xt[:, rb0 * 512: rb0 * 512 + rw0],
                            start=(k == 0), stop=(k == KC - 1),
                            tile_position=(0, 0), skip_group_check=True,
                        )
                        rb1 = pr * 2 + 1
                        if rb1 < nrb:
                            rw1 = min(512, w - rb1 * 512)
                            nc.tensor.matmul(
                                out=psums[pr][HID:2 * HID, :rw1],
                                lhsT=w1t[:, k * HID:(k + 1) * HID],
                                rhs=xt[:, rb1 * 512: rb1 * 512 + rw1],
                                start=(k == 0), stop=(k == KC - 1),
                                tile_position=(0, HID), skip_group_check=True,
                            )
                for pr in range(npair):
                    for half in range(2):
                        rb = pr * 2 + half
                        if rb >= nrb:
                            continue
                        rw = min(512, w - rb * 512)
                        stg = io2.tile([HID, 512], F32, tag="stg")
                        nc.vector.tensor_copy(
                            out=stg[:, :rw],
                            in_=psums[pr][half * HID:(half + 1) * HID, :rw],
                        )
                        for b in range(rw // P):
                            rt = (c0 + rb * 512 + b * P) // P
                            tp = pstp.tile([P, HID], F32, tag="tp")
                            nc.tensor.transpose(
                                out=tp[:], in_=stg[:, b * P:(b + 1) * P],
                                identity=ident[0:HID, 0:HID],
                            )
                            # scalar2 undoes the x64 host prescale of W1
                            nc.vector.tensor_scalar(
                                out=acc1[:, rt * HID:(rt + 1) * HID],
                                in0=tp[:], scalar1=dvt[:, rt:rt + 1],
                                scalar2=0.015625, op0=AL.mult, op1=AL.mult,
                            )
                nc.sync.dma_start(
                    out=shard1[c0:c1, :].rearrange("(rt p) c -> p rt c", p=P),
                    in_=acc1[:, c0 // P * HID:c1 // P * HID].bitcast(F32)
                        .rearrange("p (rt c) -> p rt c", rt=(c1 - c0) // P),
                )  # f8 cols /4 under f32 bitcast -> 16 per tile
            nc.sync.dma_start(out=shard1[NRANKS:TROWS, :], in_=zt[:])
            ag_full(shard1, table1)

            # ---------------- GCN aggregation + next-layer matmul, pipelined.
            # after_chunk(r0, r1) runs the dependent matmul + shard write for
            # the finished rank range so it overlaps the next chunk's gathers;
            # the (single) AllGather is emitted by the caller afterwards.
            def gcn_aggregate(table, btile, hT, after_chunk):
                for (g_lo, g_hi, r0, r1) in chunks:
                    for gi in range(g_lo, g_hi):
                        (t0, T, Dg, base, Dg2, npass, base2) = groups[gi]
                        slot = slotp.tile([P, SLOTCAP * 16], F32, tag="slot")
                        for pp in range(npass):
                            op = AL.bypass if pp == 0 else AL.add
                            pbase = base2 + pp * T * Dg2 if KACC > 1 else base
                            for cc0 in range(0, T * Dg2, GCHUNK):
                                cw = min(GCHUNK, T * Dg2 - cc0)
                                gq(nc.gpsimd.indirect_dma_start(
                                    out=slot[:, cc0 * 16: (cc0 + cw) * 16],
                                    out_offset=None,
                                    in_=table.opt(),
                                    in_offset=bass.IndirectOffsetOnAxis(
                                        ap=idxt2[:, pbase + cc0: pbase + cc0 + cw],
                                        axis=0
                                    ),
                                    compute_op=op,
                                ))
                        sv8 = slot[:, : T * Dg2 * 16].bitcast(F8D).rearrange(
                            "p (t d f) -> p t d f", t=T, d=Dg2
                        )
                        # first tree level casts f8 pairs into bf16 accumulator
                        hh = Dg2 // 2
                        nd = Dg2 - hh
                        svbt = slotp.tile([P, 80 * HID], BF16, tag="slotb", bufs=2)
                        sv = svbt[:, : T * nd * HID].rearrange(
                            "p (t d f) -> p t d f", t=T, d=nd
                        )
                        if hh > 0:
                            nc.vector.tensor_add(
                                out=sv[:, :, 0:hh, :], in0=sv8[:, :, 0:hh, :],
                                in1=sv8[:, :, nd:Dg2, :],
                            )
                        if nd > hh:
                            nc.vector.tensor_copy(
                                out=sv[:, :, hh:nd, :], in_=sv8[:, :, hh:nd, :],
                            )
                        _tree_reduce(nc, sv, nd)
                        hbuf = io2.tile([P, MAXT * HID], F32, tag="hbuf")
                        for tt in range(T):
                            nc.vector.scalar_tensor_tensor(
                                out=hbuf[:, tt * HID:(tt + 1) * HID],
                                in0=sv[:, tt, 0, :],
                                scalar=dvt[:, t0 + tt: t0 + tt + 1],
                                in1=btile[:],
                                op0=AL.mult, op1=AL.add,
                            )
                        nc.scalar.activation(
                            out=hbuf[:, : T * HID], in_=hbuf[:, : T * HID],
                            func=AF.Relu
                        )
                        for tt in range(T):
                            tph = pstp.tile([HID, P], F32, tag="tp")
                            nc.tensor.transpose(
                                out=tph[:], in_=hbuf[:, tt * HID:(tt + 1) * HID],
                                identity=ident[:],
                            )
                            # Relu == exact copy here (hbuf is post-relu, >=0)
                            # and its ACT path is already exercised on HW.
                            nc.scalar.activation(
                                out=hT[:, (t0 + tt) * P:(t0 + tt + 1) * P],
                                in_=tph[:], func=AF.Relu,
                            )
                    after_chunk(r0, r1)

            # ---------------- L2 matmul on a finished h1T rank range
            def mm2_chunk(r0, r1):
                for c0 in range(r0, r1, 512):
                    w = min(512, r1 - c0)
                    ps2 = psmm.tile([HID, 512], F32, tag="mm")
                    nc.tensor.matmul(
                        out=ps2[:, :w], lhsT=w2t[:], rhs=h1T[:, c0:c0 + w],
                        start=True, stop=True,
                    )
                    stg = io2.tile([HID, 512], F32, tag="stg")
                    nc.vector.tensor_copy(out=stg[:, :w], in_=ps2[:, :w])
                    for b in range(w // P):
                        rt = (c0 + b * P) // P
                        tp = pstp.tile([P, HID], F32, tag="tp")
                        nc.tensor.transpose(
                            out=tp[:], in_=stg[:, b * P:(b + 1) * P],
                            identity=ident[0:HID, 0:HID],
                        )
                        nc.vector.tensor_scalar(
                            out=acc2[:, rt * HID:(rt + 1) * HID],
                            in0=tp[:], scalar1=dvt[:, rt:rt + 1], scalar2=None,
                            op0=AL.mult,
                        )
                nc.sync.dma_start(
                    out=shard2[r0:r1, :].rearrange("(rt p) c -> p rt c", p=P),
                    in_=acc2[:, r0 // P * HID:r1 // P * HID].bitcast(F32)
                        .rearrange("p (rt c) -> p rt c", rt=(r1 - r0) // P),
                )

            # ---------------- GAT node transform on a finished h2T rank range
            def mm3_chunk(r0, r1):
                for c0 in range(r0, r1, 512):
                    w = min(512, r1 - c0)
                    ps3 = psmm.tile([18, 512], F32, tag="mm")
                    nc.tensor.matmul(
                        out=ps3[:, :w], lhsT=wgt[:], rhs=h2T[:, c0:c0 + w],
                        start=True, stop=True,
                    )
                    stg = io2.tile([18, 512], F32, tag="stg3")
                    nc.vector.tensor_copy(out=stg[:, :w], in_=ps3[:, :w])
                    for b in range(w // P):
                        rt = (c0 + b * P) // P
                        tp = pstp.tile([P, 18], F32, tag="tp")
                        nc.tensor.transpose(
                            out=tp[:], in_=stg[:, b * P:(b + 1) * P],
                            identity=ident[0:18, 0:18],
                        )
                        nc.vector.tensor_copy(
                            out=acc3[:, rt * GW: rt * GW + GW], in_=tp[:, 0:GW]
                        )
                        nc.vector.tensor_copy(
                            out=adst[:, rt * 2: rt * 2 + 2], in_=tp[:, 16:18]
                        )
                nc.sync.dma_start(
                    out=shard3[r0:r1, :].rearrange("(rt p) c -> p rt c", p=P),
                    in_=acc3[:, r0 // P * GW:r1 // P * GW].bitcast(F32)
                        .rearrange("p (rt c) -> p rt c", rt=(r1 - r0) // P),
                )

            acc2 = cp.tile([P, NT * HID], F8D, tag="acc2")
            acc3 = cp.tile([P, NT * GW], F8D, tag="acc3")

            # ---------------- L1 aggregation -> h1T (mm2 pipelined in)
            gcn_aggregate(table1, b1t, h1T, mm2_chunk)
            nc.sync.dma_start(out=shard2[NRANKS:TROWS, :], in_=zt[:])
            ag_full(shard2, table2)

            # ---------------- L2 aggregation -> h2T (mm3 pipelined in)
            gcn_aggregate(table2, b2t, h2T, mm3_chunk)
            nc.sync.dma_start(out=shard3[NRANKS:TROWS, :], in_=gpad[:].bitcast(F32))
            ag_full(shard3, table3)

            # ---------------- Stage J: GAT aggregation (heads fused)
            adv = adst[:].rearrange("p (t h) -> p t h", h=2)
            oaccv = oacc[:].rearrange("p (t c) -> p t c", c=14)
            for (t0, T, Dg, base, Dg2, npass, base2) in groups:
                slot = slotp.tile([P, SLOTCAP * 4], F32, tag="slot3", name="slot3")
                for cc0 in range(0, T * Dg, GCHUNK):
                    cw = min(GCHUNK, T * Dg - cc0)
                    gq(nc.gpsimd.indirect_dma_start(
                        out=slot[:, cc0 * 4: (cc0 + cw) * 4],
                        out_offset=None,
                        in_=table3.opt(),
                        in_offset=bass.IndirectOffsetOnAxis(
                            ap=idxt[:, base + cc0: base + cc0 + cw], axis=0
                        ),
                    ))
                sv = slot[:, : T * Dg * 4].bitcast(F8D).rearrange(
                    "p (t d f) -> p t d f", t=T, d=Dg
                )
                TDg = T * Dg
                # e = a_src[src] + a_dst[dst], head-major flat: head h at
                # cols [h*TDg, (h+1)*TDg), each viewed [P, T, Dg]
                ev = io2.tile([P, SLOTCAP * 2], F32, tag="ev")
                for h in range(HEADS):
                    nc.vector.tensor_tensor(
                        out=ev[:, h * TDg:(h + 1) * TDg].rearrange(
                            "p (t d) -> p t d", t=T),
                        in0=sv[:, :, :, 14 + h],
                        in1=adv[:, t0:t0 + T, h][:, :, None].to_broadcast(
                            [P, T, Dg]),
                        op=AL.add,
                    )
                # leaky relu 0.2 both heads at once (device Lrelu is broken)
                et = io2.tile([P, SLOTCAP * 2], F32, tag="et")
                nc.vector.tensor_scalar(
                    out=et[:, : 2 * TDg], in0=ev[:, : 2 * TDg],
                    scalar1=0.2, scalar2=None, op0=AL.mult,
                )
                nc.vector.tensor_max(
                    out=ev[:, : 2 * TDg], in0=ev[:, : 2 * TDg],
                    in1=et[:, : 2 * TDg]
                )
                # segment max per (head, tile), subtract (baseline 3-D forms)
                mx = io2.tile([P, MAXT * 2], F32, tag="mx")
                for h in range(HEADS):
                    e3 = ev[:, h * TDg:(h + 1) * TDg].rearrange(
                        "p (t d) -> p t d", t=T)
                    nc.vector.tensor_reduce(
                        out=mx[:, h * T:(h + 1) * T], in_=e3, axis=AX.X,
                        op=AL.max,
                    )
                    nc.vector.tensor_tensor(
                        out=e3, in0=e3,
                        in1=mx[:, h * T:(h + 1) * T][:, :, None].to_broadcast(
                            [P, T, Dg]),
                        op=AL.subtract,
                    )
                pb = io2.tile([P, SLOTCAP * 2], BF16, tag="pb")
                nc.scalar.activation(
                    out=pb[:, : 2 * TDg], in_=ev[:, : 2 * TDg], func=AF.Exp
                )
                ss = io2.tile([P, MAXT * 2], F32, tag="ss")
                for h in range(HEADS):
                    nc.vector.tensor_reduce(
                        out=ss[:, h * T:(h + 1) * T],
                        in_=pb[:, h * TDg:(h + 1) * TDg].rearrange(
                            "p (t d) -> p t d", t=T),
                        axis=AX.X, op=AL.add,
                    )
                rc = io2.tile([P, MAXT * 2], F32, tag="rc")
                nc.vector.reciprocal(out=rc[:, : 2 * T], in_=ss[:, : 2 * T])
                # replicate alpha to 14 wide -> single weighted tree-reduce
                p14 = io2.tile([P, SLOTCAP * 14], BF16, tag="p14")
                p14v = p14[:, : TDg * 14].rearrange(
                    "p (t d c) -> p t d c", t=T, d=Dg)
                for h in range(HEADS):
                    nc.vector.tensor_copy(
                        out=p14v[:, :, :, h * CLS:(h + 1) * CLS],
                        in_=pb[:, h * TDg:(h + 1) * TDg].rearrange(
                            "p (t d) -> p t d", t=T)[:, :, :, None].to_broadcast(
                            [P, T, Dg, CLS]),
                    )
                wb = io2.tile([P, SLOTCAP * 14], BF16, tag="wb")
                wv = wb[:, : TDg * 14].rearrange(
                    "p (t d c) -> p t d c", t=T, d=Dg)
                nc.vector.tensor_tensor(
                    out=wv, in0=sv[:, :, :, 0:14],
                    in1=p14v, op=AL.mult,
                )
                _tree_reduce(nc, wv, Dg)
                for h in range(HEADS):
                    nc.vector.tensor_tensor(
                        out=oaccv[:, t0:t0 + T, h * CLS:(h + 1) * CLS],
                        in0=wv[:, :, 0, h * CLS:(h + 1) * CLS],
                        in1=rc[:, h * T:(h + 1) * T][:, :, None].to_broadcast(
                            [P, T, CLS]),
                        op=AL.mult,
                    )
            # + bg, log_softmax
            nc.vector.tensor_tensor(
                out=oaccv, in0=oaccv,
                in1=bgt[:][:, None, :].to_broadcast([P, NT, 14]), op=AL.add,
            )
            mx2 = cp.tile([P, NT], F32, tag="mx2")
            nc.vector.tensor_reduce(out=mx2[:], in_=oaccv, axis=AX.X, op=AL.max)
            nc.vector.tensor_tensor(
                out=oaccv, in0=oaccv,
                in1=mx2[:][:, :, None].to_broadcast([P, NT, 14]), op=AL.subtract,
            )
            ex = cp.tile([P, NT * 14], F32, tag="ex")
            nc.scalar.activation(out=ex[:], in_=oacc[:], func=AF.Exp)
            ssum = cp.tile([P, NT], F32, tag="ssum")
            nc.vector.tensor_reduce(
                out=ssum[:], in_=ex[:].rearrange("p (t c) -> p t c", c=14),
                axis=AX.X, op=AL.add,
            )
            lg = cp.tile([P, NT], F32, tag="lg")
            nc.scalar.activation(out=lg[:], in_=ssum[:], func=AF.Ln)
            nc.vector.tensor_tensor(
                out=oaccv, in0=oaccv,
                in1=lg[:][:, :, None].to_broadcast([P, NT, 14]), op=AL.subtract,
            )
            nc.sync.dma_start(out=out[:], in_=oacc[:])

    nc.compile()
    return nc


def _prepare(x, edge_index, W1, b1, W2, b2, Wg, att_src, att_dst, bg):
    """Preprocess + build/caching + per-core input maps. Shared with test.py."""
    perms, groups, chunks, S_gat, S_gcn, gidx, gidx2, dinv_arr = \
        _preprocess(edge_index)

    key = (S_gat, S_gcn, tuple(groups), tuple(chunks))
    if key not in _cache:
        _cache[key] = _build(groups, chunks, S_gat, S_gcn)
    nc = _cache[key]

    W1p = np.zeros((F_PAD, HID), np.float32)
    W1p[:F_IN] = W1 * 64.0  # prescale into fp8's dense range; undone on device
    w1d = np.ascontiguousarray(
        W1p.reshape(KC, P, HID).transpose(1, 0, 2).reshape(P, KC * HID)
    ).astype(F8)
    w2d = W2.astype(BF)
    wsrc = np.stack(
        [Wg[:, h * CLS:(h + 1) * CLS] @ att_src[h] for h in range(HEADS)], axis=1
    )
    wdst = np.stack(
        [Wg[:, h * CLS:(h + 1) * CLS] @ att_dst[h] for h in range(HEADS)], axis=1
    )
    wgd = np.concatenate([Wg, wsrc, wdst], axis=1).astype(BF)  # [64, 18]
    b1d = np.ascontiguousarray(np.broadcast_to(b1, (P, HID))).astype(BF)
    b2d = np.ascontiguousarray(np.broadcast_to(b2, (P, HID))).astype(BF)
    bgd = np.ascontiguousarray(np.broadcast_to(bg, (P, 14))).astype(np.float32)

    in_maps = []
    for k in range(NC):
        xk = x[k * NPC:(k + 1) * NPC][perms[k]]  # [NPC, F_IN]
        xTk = np.zeros((F_PAD, NRANKS), F8)
        xTk[:F_IN, :NPC] = xk.T.astype(F8)
        in_maps.append({
            "xT": xTk,
            "w1": w1d, "w2": w2d, "wg": wgd,
            "b1": b1d, "b2": b2d, "bg": bgd,
            "dinv": dinv_arr[k],
            "gidx": gidx[k],
            **({"gidx2": gidx2[k]} if KACC > 1 else {}),
        })
    return nc, in_maps, perms


def _unshard(res, perms):
    outf = np.empty((N, 14), np.float32)
    for k in range(NC):
        ok = res.results[k]["out"].reshape(P, NT, 14).transpose(1, 0, 2) \
            .reshape(NRANKS, 14)[:NPC]  # rank order
        outf[k * NPC + perms[k]] = ok
    return outf


def kernel(x, edge_index, W1, b1, W2, b2, Wg, att_src, att_dst, bg, **_):
    x = np.asarray(x)
    edge_index = np.asarray(edge_index)
    W1 = np.asarray(W1, np.float32)
    b1 = np.asarray(b1, np.float32)
    W2 = np.asarray(W2, np.float32)
    b2 = np.asarray(b2, np.float32)
    Wg = np.asarray(Wg, np.float32)
    att_src = np.asarray(att_src, np.float32)
    att_dst = np.asarray(att_dst, np.float32)
    bg = np.asarray(bg, np.float32)

    nc, in_maps, perms = _prepare(x, edge_index, W1, b1, W2, b2, Wg,
                                  att_src, att_dst, bg)
    res = run_bass_kernel_spmd(nc, in_maps, core_ids=list(range(NC)))
    return _unshard(res, perms)


if __name__ == "__main__":
    rng = np.random.default_rng(0)
    x = rng.standard_normal((N, F_IN)).astype(np.float32)
    ei = rng.integers(0, N, (2, 1600000)).astype(np.int32)
    W1 = rng.standard_normal((F_IN, HID)).astype(np.float32) * 0.02
    W2 = rng.standard_normal((HID, HID)).astype(np.float32) * 0.1
    Wg = rng.standard_normal((HID, HEADS * CLS)).astype(np.float32) * 0.1
    o = kernel(
        x, ei, W1, np.zeros(HID, np.float32), W2, np.zeros(HID, np.float32),
        Wg, rng.standard_normal((HEADS, CLS)).astype(np.float32) * 0.1,
        rng.standard_normal((HEADS, CLS)).astype(np.float32) * 0.1,
        np.zeros(HEADS * CLS, np.float32),
    )
    print("kernel output", o.shape, o[:2])
